# revision 1
# baseline (speedup 1.0000x reference)
"""Causal-attention (QKV projection + softmax(QK^T/sqrt(d))V) on 8 trn2 cores.

Contract: kernel(x, Wq, Wk, Wv) takes FULL inputs
  x [4, 4096, 768] f32, Wq/Wk/Wv [768, 128] f32
and returns the FULL output [4, 4096, 128] f32.

Sharding: 2 cores per batch. Core with parity h in {0,1} of batch b owns query
rows h::2 (perfect causal load balance). The host permutes the per-core input
to xT_p = concat(x[b, h::2], x[b, 1-h::2]).T so one compiled SPMD program runs
on every core; causality is enforced with two per-core [128,128] triangular
additive-mask tiles applied only on the diagonal 128-key blocks.

v2 changes vs baseline (105.9us):
  - scores/exp/sacc/AV trimmed to the causal column range per diagonal tile
  - mask matmuls shrunk from N=512 to N=128 (triangle block only)
  - kt/qt copies on ScalarE between exp phases, v/po/sacc on DVE,
    v evacuated 4 chunks per [128,512] copy
  - x loaded in 8 column-wave DMAs split across sync+gpsimd queues
  - longer PE warmup bridging the input-DMA wait (HAM + pstate ramp)
  - projections of tile t+1 interleaved into tile t's pair loop
"""
import numpy as np

import concourse.bass as bass
import concourse.mybir as mybir
import concourse.tile as tile_mod
from concourse.tile import ScopedClock, VectorClock
from concourse.tile_sem_assignment import N_PROCS
from concourse.bass_utils import run_bass_kernel_spmd

f32 = mybir.dt.float32
f16 = mybir.dt.float16

B, S, D_IN, D = 4, 4096, 768, 128
N_DIN = D_IN // 128  # 6
TQ = 512             # queries per q-tile
NQ = S // 2          # queries per core
N_QT = NQ // TQ      # 4 q-tiles
SCALE = 1.0 / np.sqrt(np.float32(D))
AF = mybir.ActivationFunctionType
N_WARM = 14

# ---------------------------------------------------------------------------
# Workarounds: the walrus build in this container accepts only ONE sync-wait
# command per instruction. TileContext's exit drain carries one wait per
# active proc, and Tile's sem assignment emits multi-wait instructions.
# Split both onto single-wait carrier instructions.
# ---------------------------------------------------------------------------


def _split_drain_and_barrier(self, tick_clock, wait_clock):
    gc = tick_clock.global_clock
    engs = [self.nc.sync, self.nc.scalar, self.nc.vector, self.nc.tensor]
    k = 0
    for p in range(N_PROCS):
        if gc[p] == 0:
            continue
        vc = VectorClock([gc[q] if q == p else 0 for q in range(N_PROCS)])
        d = engs[k % len(engs)].drain()
        k += 1
        wait_clock.add_sem_waits(d.ins, ScopedClock({None: vc}))
    self.nc.all_engine_barrier()
    assert self.sems is not None
    popped = self.nc._tile_sem_poison_stack.pop()
    assert popped is self._sem_poison
    self.nc.clear_and_free_semaphores(list(self.sems.allocated().values()))
    self.nc.all_engine_barrier()


tile_mod.TileContext._drain_and_barrier = _split_drain_and_barrier


def _split_waits(nc, max_waits=1):
    for fn in nc.m.functions:
        for bb in fn.blocks:
            insts = bb.instructions
            if not any(
                i.sync_info and i.sync_info.on_wait
                and len(i.sync_info.on_wait) > max_waits
                for i in insts
            ):
                continue
            new = []
            for inst in insts:
                si = inst.sync_info
                ow = list(si.on_wait) if si and si.on_wait else []
                if len(ow) > max_waits:
                    excess, keep = ow[:-max_waits], ow[-max_waits:]
                    for j, w in enumerate(excess):
                        new.append(
                            mybir.InstEventSemaphore(
                                name=f"{inst.name}-wsplit{j}",
                                engine=inst.engine,
                                ins=[],
                                outs=[],
                                sync_info=mybir.SyncInfo(
                                    on_wait=[w], on_update=[]
                                ),
                            )
                        )
                    inst.sync_info = mybir.SyncInfo(
                        on_wait=keep, on_update=list(si.on_update or [])
                    )
                new.append(inst)
            bb.instructions = new


# ---------------------------------------------------------------------------
# Device program
# ---------------------------------------------------------------------------


def _build():
    nc = bass.Bass()
    xT = nc.declare_dram_parameter("xT", [D_IN, S], f16, isOutput=False)
    W = nc.declare_dram_parameter("W", [128, N_DIN * 3 * D], f16, isOutput=False)
    mask = nc.declare_dram_parameter("mask", [128, 3 * 128], f16, isOutput=False)
    out_num = nc.declare_dram_parameter("out_num", [D, NQ], f32, isOutput=True)
    out_den = nc.declare_dram_parameter("out_den", [128, 2 * NQ], f16, isOutput=True)

    with tile_mod.TileContext(nc) as tc:
        with (
            tc.tile_pool(name="persist", bufs=1) as persist,
            tc.tile_pool(name="work", bufs=6) as work,
            tc.tile_pool(name="sacc_p", bufs=2) as sacc_p,
            tc.tile_pool(name="outp", bufs=2) as outp,
            tc.tile_pool(name="ps_big", bufs=2, space="PSUM") as ps_big,
            tc.tile_pool(name="ps_out", bufs=2, space="PSUM") as ps_out,
            tc.tile_pool(name="ps_sml", bufs=2, space="PSUM") as ps_sml,
        ):
            x_all = persist.tile([128, N_DIN, S], f16, tag="x_all")
            w_all = persist.tile([128, N_DIN * 3 * D], f16, tag="w_all")
            m_all = persist.tile([128, 3 * 128], f16, tag="m_all")
            kt_sb = [persist.tile([128, 512], f16, tag=f"kt{c}", name=f"kt{c}")
                     for c in range(S // 512)]
            qt_sb = [persist.tile([128, TQ], f16, tag=f"qt{t}", name=f"qt{t}")
                     for t in range(N_QT)]
            # v_sb[g] holds key tiles 4g..4g+3 in [keys, d] layout cols 128j
            v_sb = [persist.tile([128, 512], f16, tag=f"v{g}", name=f"v{g}")
                    for g in range(S // 512)]

            # W host layout: [K block | V block | Q block], di-major inside
            def w_k(di):
                return w_all[:, 128 * di:128 * (di + 1)]

            def w_v(di):
                return w_all[:, 768 + 128 * di:768 + 128 * (di + 1)]

            def w_q(di):
                return w_all[:, 1536 + 128 * di:1536 + 128 * (di + 1)]

            tri = [m_all[:, 0:128], m_all[:, 128:256]]  # half1, half2
            ident = m_all[:, 256:384]

            # input DMAs. W blocks (K first — needed earliest) + mask on
            # sync; x on gpsimd: wave(0,0) split per-di so the t=0
            # projections can start on the first 128 rows of x, the rest as
            # whole column waves in priority order (per-queue FIFO).
            nc.sync.dma_start(out=w_all[:, 0:768], in_=W[:, 0:768])
            nc.sync.dma_start(out=w_all[:, 768:1536], in_=W[:, 768:1536])
            nc.sync.dma_start(out=w_all[:, 1536:2304], in_=W[:, 1536:2304])
            nc.sync.dma_start(out=m_all[:], in_=mask[:])
            xsrc = xT.rearrange("(d p) c -> p d c", p=128)
            half = S // 2

            # PE pre-warm bridging the input-DMA wait: HAM un-throttles after
            # ~3.4us of sustained activity and pstate ramps after ~3us, so
            # keep the PE busy until the first wave lands.
            warm_sb = persist.tile([128, 512], f16, tag="warm")
            nc.gpsimd.memset(warm_sb[:], 0.0)

            def x_wave(t, h):
                lo = 512 * t + half * h
                nc.gpsimd.dma_start(
                    out=x_all[:, :, lo:lo + 512], in_=xsrc[:, :, lo:lo + 512]
                )

            for di in range(N_DIN):  # wave(0,0) per-di
                nc.gpsimd.dma_start(
                    out=x_all[:, di, 0:512], in_=xsrc[:, di, 0:512]
                )
            x_wave(0, 1)
            for t in (1, 2, 3):
                x_wave(t, 0)
                x_wave(t, 1)

            for i in range(N_WARM):
                psw = ps_sml.tile([128, 512], f32, tag="sml", name=f"warm{i}")
                nc.tensor.matmul(
                    psw[:], lhsT=warm_sb[:, 0:128], rhs=warm_sb[:],
                    start=True, stop=True,
                )

            def x_cols(di, c0, c1):
                return x_all[:, di, c0:c1]

            def project_kt(c):
                ps = ps_sml.tile([128, 512], f32, tag="sml", name=f"pkt{c}")
                for di in range(N_DIN):
                    nc.tensor.matmul(
                        ps[:],
                        lhsT=w_k(di),
                        rhs=x_cols(di, 512 * c, 512 * (c + 1)),
                        start=(di == 0),
                        stop=(di == N_DIN - 1),
                    )
                nc.vector.tensor_copy(kt_sb[c][:], ps[:])

            def project_qt(t):
                ps = ps_sml.tile([128, 512], f32, tag="sml", name=f"pqt{t}")
                for di in range(N_DIN):
                    nc.tensor.matmul(
                        ps[:],
                        lhsT=w_q(di),
                        rhs=x_cols(di, TQ * t, TQ * (t + 1)),
                        start=(di == 0),
                        stop=(di == N_DIN - 1),
                    )
                nc.vector.tensor_copy(qt_sb[t][:], ps[:])

            def project_v_group(g):
                # key tiles 4g..4g+3 -> one [128,512] psum tile, one DVE evac
                ps = ps_sml.tile([128, 512], f32, tag="sml", name=f"pv{g}")
                for j in range(4):
                    k = 4 * g + j
                    for di in range(N_DIN):
                        nc.tensor.matmul(
                            ps[:, 128 * j:128 * (j + 1)],
                            lhsT=x_cols(di, 128 * k, 128 * (k + 1)),
                            rhs=w_v(di),
                            start=(di == 0),
                            stop=(di == N_DIN - 1),
                        )
                nc.vector.tensor_copy(v_sb[g][:], ps[:])

            def project_t0_interleaved():
                # kt(0) + qt(0) accumulated per-di as each 128-row slab of x
                # arrives (two open PSUM groups in separate banks; more would
                # alias: start=True pending-zeroes the whole 2KB bank).
                psk = ps_sml.tile([128, 512], f32, tag="sml", name="pkt0")
                psq = ps_big.tile([128, 2 * TQ], f32, tag="big", name="pqt0")
                for di in range(N_DIN):
                    st, sp = di == 0, di == N_DIN - 1
                    nc.tensor.matmul(psk[:], lhsT=w_k(di),
                                     rhs=x_cols(di, 0, 512), start=st, stop=sp)
                    nc.tensor.matmul(psq[:, 0:512], lhsT=w_q(di),
                                     rhs=x_cols(di, 0, 512), start=st, stop=sp)
                    if di < N_DIN - 1:
                        # keep the PE busy across the DMA-paced slab waits so
                        # HAM doesn't re-throttle
                        psw = ps_out.tile([128, TQ], f32, tag="out",
                                          name=f"warmd{di}")
                        nc.tensor.matmul(
                            psw[:], lhsT=warm_sb[:, 0:128], rhs=warm_sb[:],
                            start=True, stop=True,
                        )
                nc.vector.tensor_copy(kt_sb[0][:], psk[:])
                nc.vector.tensor_copy(qt_sb[0][:], psq[:, 0:512])
                psw = ps_out.tile([128, TQ], f32, tag="out", name="warmt0")
                nc.tensor.matmul(psw[:], lhsT=warm_sb[:, 0:128], rhs=warm_sb[:],
                                 start=True, stop=True)
                project_v_group(0)

            def proj_ops(t):
                """Projection op closures for q-tile t (emitted one tile early).

                For t == N_QT-1 the diagonal-chunk projections are deferred
                into that tile's own pair loop (late_ops) so the PE has
                filler work while ScalarE paces the final exp chain.
                """
                if t >= N_QT:
                    return []
                if t == N_QT - 1:
                    return [
                        lambda: project_qt(t),
                        lambda: project_kt(t),
                    ]
                return [
                    lambda: project_kt(t),
                    lambda: project_kt(N_QT + t),
                    lambda: project_v_group(t),
                    lambda: project_v_group(N_QT + t),
                    lambda: project_qt(t),
                ]

            def fill_ops(t):
                """Fill for tile t's pair loop: finish t's own other-parity
                projections (t=0 only), then prefetch tile t+1."""
                if t == 0:
                    return [
                        lambda: project_kt(N_QT),
                        lambda: project_v_group(N_QT),
                    ] + proj_ops(1)
                return proj_ops(t + 1)

            def late_ops(t):
                """(position, op) pairs injected into tile t's own loop."""
                if t != N_QT - 1:
                    return []
                return [
                    (3, lambda: project_v_group(t)),
                    (9, lambda: project_kt(N_QT + t)),
                    (11, lambda: project_v_group(N_QT + t)),
                ]

            project_t0_interleaved()

            n_kt_half = NQ // 128  # 16

            for t in range(N_QT):
                pairs = [2 * j for j in range(2 * (t + 1))] + [
                    n_kt_half + 2 * j for j in range(2 * (t + 1))
                ]
                n = len(pairs)

                def pair_info(kp):
                    half2 = kp >= n_kt_half
                    rel = kp - n_kt_half if half2 else kp
                    diag = 4 * t <= rel < 4 * t + 4
                    los = (
                        [128 * (rel - 4 * t), 128 * (rel - 4 * t + 1)]
                        if diag else [0, 0]
                    )
                    return half2, diag, los

                def emit_scores(kp, name):
                    half2, diag, los = pair_info(kp)
                    ps = ps_big.tile([128, 2 * TQ], f32, tag="big", name=name)
                    for s_ in (0, 1):
                        kt = kp + s_
                        lo = los[s_]
                        nc.tensor.matmul(
                            ps[:, TQ * s_ + lo:TQ * (s_ + 1)],
                            lhsT=kt_sb[kt // 4][:, 128 * (kt % 4):128 * (kt % 4 + 1)],
                            rhs=qt_sb[t][:, lo:TQ],
                            start=True,
                            stop=not diag,
                            skip_group_check=diag,
                        )
                        if diag:
                            nc.tensor.matmul(
                                ps[:, TQ * s_ + lo:TQ * s_ + lo + 128],
                                lhsT=ident,
                                rhs=tri[1 if half2 else 0],
                                start=False,
                                stop=True,
                                skip_group_check=True,
                            )
                    return ps

                # software pipeline: scores one pair ahead of exp/AV
                fill = fill_ops(t)
                fill_done = 0
                late = list(late_ops(t))

                po = ps_out.tile([128, TQ], f32, tag="out", name=f"po{t}")
                sacc = sacc_p.tile([128, 2 * TQ], f16, tag="sacc", name=f"sacc{t}")

                def sacc_accum(dst, src, first):
                    if first:
                        nc.vector.tensor_copy(dst, src)
                    else:
                        nc.vector.tensor_add(dst, dst, src)

                while late and late[0][0] <= 0:
                    late.pop(0)[1]()
                ps_q = [emit_scores(pairs[0], f"s{t}_0")]
                first_av = True
                for i, kp in enumerate(pairs):
                    # interleave next tile's projections
                    want = ((i + 1) * len(fill)) // n
                    while fill_done < want:
                        fill[fill_done]()
                        fill_done += 1
                    while late and late[0][0] <= i + 1:
                        late.pop(0)[1]()
                    if i + 1 < n:
                        ps_q.append(emit_scores(pairs[i + 1], f"s{t}_{i + 1}"))
                    ps = ps_q.pop(0)
                    half2, diag, los = pair_info(kp)
                    pt = work.tile([128, 2 * TQ], f16, tag="pt",
                                   name=f"p{t}_{kp}")
                    if diag:
                        for s_ in (0, 1):
                            lo = los[s_]
                            nc.scalar.activation(
                                pt[:, TQ * s_ + lo:TQ * (s_ + 1)],
                                ps[:, TQ * s_ + lo:TQ * (s_ + 1)],
                                AF.Exp, scale=float(SCALE),
                            )
                            sacc_accum(
                                sacc[:, TQ * s_ + lo:TQ * (s_ + 1)],
                                pt[:, TQ * s_ + lo:TQ * (s_ + 1)],
                                i == 0,
                            )
                    else:
                        nc.scalar.activation(
                            pt[:], ps[:], AF.Exp, scale=float(SCALE)
                        )
                        sacc_accum(sacc[:], pt[:], i == 0)
                    for s_ in (0, 1):
                        kt = kp + s_
                        lo = los[s_]
                        nc.tensor.matmul(
                            po[:, lo:TQ],
                            lhsT=v_sb[kt // 4][:, 128 * (kt % 4):128 * (kt % 4 + 1)],
                            rhs=pt[:, TQ * s_ + lo:TQ * (s_ + 1)],
                            start=first_av,
                            stop=(i == n - 1 and s_ == 1),
                            skip_group_check=True,
                        )
                        first_av = False
                ob = outp.tile([128, TQ], f32, tag="ob", name=f"ob{t}")
                nc.scalar.activation(ob[:], po[:], AF.Copy)
                # den's last producer finishes ~1.4us before ob; issue den
                # first so the sync queue's FIFO doesn't stall it behind num
                nc.sync.dma_start(
                    out=out_den[:, 2 * TQ * t:2 * TQ * (t + 1)], in_=sacc[:]
                )
                nc.sync.dma_start(out=out_num[:, TQ * t:TQ * (t + 1)], in_=ob[:])
    _split_waits(nc)
    return nc


_NC_CACHE = []


def _get_nc():
    if not _NC_CACHE:
        _NC_CACHE.append(_build())
    return _NC_CACHE[0]


def _host_inputs(x, Wq, Wk, Wv):
    # device W layout: [K block | V block | Q block], di-major inside a block
    def blk(M):
        return M.astype(np.float16).reshape(N_DIN, 128, D).transpose(1, 0, 2)

    W = np.ascontiguousarray(
        np.concatenate([blk(Wk), blk(Wv), blk(Wq)], axis=1)
        .reshape(128, N_DIN * 3 * D)
    )
    u = np.arange(128)[:, None]
    i = np.arange(128)[None, :]
    masks = {}
    for h in (0, 1):
        tri1 = (u <= i).astype(np.float32)          # own-parity half
        tri2 = (u <= i - 1 + h).astype(np.float32)  # other-parity half
        ma = np.concatenate(
            [(tri1 - 1.0) * 1000.0, (tri2 - 1.0) * 1000.0,
             np.eye(128, dtype=np.float32)], axis=1
        )
        masks[h] = np.ascontiguousarray(ma).astype(np.float16)
    in_maps = []
    for c in range(2 * B):
        b, h = divmod(c, 2)
        xp = np.concatenate([x[b, h::2], x[b, 1 - h::2]], axis=0)  # [S, 768]
        xT_p = np.ascontiguousarray(xp.T.astype(np.float16))  # [768, S]
        in_maps.append({"xT": xT_p, "W": W, "mask": masks[h]})
    return in_maps


def kernel(x, Wq, Wk, Wv):
    x = np.asarray(x, np.float32)
    Wq = np.asarray(Wq, np.float32)
    Wk = np.asarray(Wk, np.float32)
    Wv = np.asarray(Wv, np.float32)
    nc = _get_nc()
    in_maps = _host_inputs(x, Wq, Wk, Wv)
    res = run_bass_kernel_spmd(nc, in_maps, list(range(2 * B)))
    out = np.empty((B, S, D), np.float32)
    for c in range(2 * B):
        b, h = divmod(c, 2)
        num = res.results[c]["out_num"]  # [128, NQ] f32
        sacc = res.results[c]["out_den"].astype(np.float32)  # [128, 2*NQ]
        sacc[:, TQ:TQ + 128] = 0.0  # t=0 s_=1 cols [0,128): never written
        s3 = sacc.reshape(128, NQ // TQ, 2, TQ)
        den = s3.sum(axis=(0, 2)).reshape(NQ)
        out[b, h::2, :] = (num / den[None, :]).T
    return out

